# revision 1
# baseline (speedup 1.0000x reference)
"""Trainium2 Bass kernel: cached multi-head self-attention decoder block.

Per-core (batch-parallel, B=8 -> 8 cores) computation for batch b:
  q  = x @ Wq + bq        (kept transposed: qT [NS, T], pre-scaled by HD^-0.5)
  kn = x @ Wk             (kept transposed: knT [NS, T])
  vn = x @ Wv + bv        (natural [T, NS])
  k  = concat(cache_kT, knT)  [NS, S+T]   (head-dim on partitions)
  scoresT[h] = kh^T-slices x qh  -> [S+T, T] per head (s on partitions)
  probsT = exp(scoresT)  (no max-subtraction needed: |scores| <~ 8)
  o[h]   = vh'^T @ probsT  accumulated over s-chunks, where vh' = [vh | 1]
           -> rows 0..63 = unnormalized o^T, row 64 = softmax denominator
  wvT    = o * (1/denom broadcast)
  outT   = Wo^T @ wvT + bo
Host side transposes xT/kT inputs and outT/keyT outputs.
"""

import numpy as np
from contextlib import ExitStack

import concourse.bass as bass
import concourse.tile as tile
from concourse import bacc, mybir
from concourse.bass_utils import run_bass_kernel_spmd

F32 = mybir.dt.float32
F32R = mybir.dt.float32r
ALU = mybir.AluOpType
ACTF = mybir.ActivationFunctionType

B, T, S, NS, NH, HD = 8, 512, 2048, 1024, 16, 64
ST = S + T            # 2560
NC_CHUNKS = NS // 128  # 8 ns chunks (= head pairs)
SCN = ST // 128       # 20 s chunks (16 cache + 4 new)
SCALE2 = float(HD ** -0.5)  # fold both q and k scales into q

LAST_EXEC_NS = None
LAST_RESULTS = None


def _emit(ctx, tc, D):
    nc = tc.nc

    # ---------------- constants / persistent tiles ----------------
    const = ctx.enter_context(tc.tile_pool(name="const", bufs=1))
    bqs_t = const.tile([128, 8], F32, name="bqs_t")
    nc.sync.dma_start(bqs_t[:], D["bqs"][:, :])
    bop_t = const.tile([128, 8], F32, name="bop_t")
    nc.sync.dma_start(bop_t[:], D["bop"][:, :])
    bv_t = const.tile([1, NS], F32R, name="bv_t")
    nc.gpsimd.dma_start(bv_t[:], D["bv"][:, :])
    ones1 = const.tile([1, 128], F32R, name="ones1")
    nc.gpsimd.dma_start(ones1[:], D["ones"].ap()[0:1, 0:128])
    onesp_src = D["ones"].ap()[:, 128:136]

    xT_t = const.tile([128, 4096], F32R, name="xT_t")  # k-chunk-major cols
    nc.gpsimd.dma_start(
        xT_t[:].rearrange("p (k t) -> p k t", k=8),
        D["xT"].ap().rearrange("(k p) t -> p k t", p=128),
    )

    pers = ctx.enter_context(tc.tile_pool(name="pers", bufs=1))
    qT_t = pers.tile([128, 4096], F32R, name="qT_t")   # (x@Wq+bq)*s2, m-chunk-major
    knT_t = pers.tile([128, 4096], F32R, name="knT_t")  # (x@Wk)^T, matmul copy
    knTf_t = pers.tile([128, 4096], F32, name="knTf_t")  # (x@Wk)^T, exact f32 for keyT
    vnew_t = pers.tile([128, 4096], F32, name="vnew_t")  # x@Wv+bv natural, tc-major
    outT_t = pers.tile([128, 4096], F32, name="outT_t")

    wv_pool = ctx.enter_context(tc.tile_pool(name="wv", bufs=1))
    wv_tiles = [wv_pool.tile([128, 512], F32R, name=f"wv_{c}", tag=f"wv{c}")
                for c in range(NC_CHUNKS)]

    wo_pool = ctx.enter_context(tc.tile_pool(name="wo", bufs=1))
    wo_tiles = [wo_pool.tile([128, 1024], F32R, name=f"wo_{k}", tag=f"wo{k}")
                for k in range(8)]

    # ---------------- phase P: projections ----------------
    with ExitStack() as pctx:
        wpool = pctx.enter_context(tc.tile_pool(name="wstream", bufs=3))
        ppool = pctx.enter_context(tc.tile_pool(name="pproj", bufs=1, space="PSUM"))

        def proj_T(w_dram, out_tile, bias_col, scale, extra_f32=None):
            """out_tile[:, m*512:(m+1)*512] = ((x @ W)^T chunk m)*scale + bias."""
            pts = [ppool.tile([128, 512], F32, name=f"pp{m}", tag=f"pp{m}")
                   for m in range(8)]
            for k in range(8):
                wt = wpool.tile([128, 1024], F32R, name="wt", tag="w")
                nc.gpsimd.dma_start(wt[:], w_dram[k * 128:(k + 1) * 128, :])
                for m in range(8):
                    nc.tensor.matmul(
                        pts[m][:],
                        lhsT=wt[:, m * 128:(m + 1) * 128],
                        rhs=xT_t[:, k * 512:(k + 1) * 512],
                        start=(k == 0), stop=(k == 7),
                    )
            for m in range(8):
                dst = out_tile[:, m * 512:(m + 1) * 512]
                if bias_col is not None:
                    nc.vector.tensor_scalar(
                        dst, pts[m][:], scale, bias_col[:, m:m + 1],
                        ALU.mult, ALU.add)
                else:
                    nc.vector.tensor_copy(dst, pts[m][:])
                if extra_f32 is not None:
                    nc.vector.tensor_copy(
                        extra_f32[:, m * 512:(m + 1) * 512], pts[m][:])

        proj_T(D["Wq"].ap(), qT_t, bqs_t, SCALE2)
        proj_T(D["Wk"].ap(), knT_t, None, 1.0, extra_f32=knTf_t)
        # keyT output straight from knT
        nc.sync.dma_start(
            D["keyT"].ap().rearrange("(m p) t -> p m t", p=128),
            knTf_t[:].rearrange("p (m t) -> p m t", m=8),
        )

        # v natural: out rows t (tc chunks), cols o (2 halves); contract over ns
        vps = [ppool.tile([128, 512], F32, name=f"pv{g}", tag=f"pp{g}")
               for g in range(8)]
        for k in range(8):
            wt = wpool.tile([128, 1024], F32R, name="wt", tag="w")
            nc.gpsimd.dma_start(wt[:], D["Wv"].ap()[k * 128:(k + 1) * 128, :])
            for tc_i in range(4):
                for oh in range(2):
                    nc.tensor.matmul(
                        vps[tc_i * 2 + oh][:],
                        lhsT=xT_t[:, k * 512 + tc_i * 128:
                                  k * 512 + (tc_i + 1) * 128],
                        rhs=wt[:, oh * 512:(oh + 1) * 512],
                        start=(k == 0), stop=False,
                    )
        for tc_i in range(4):
            for oh in range(2):
                # bias row via K=1 matmul: + ones^T @ bv_slice
                nc.tensor.matmul(
                    vps[tc_i * 2 + oh][:],
                    lhsT=ones1[:],
                    rhs=bv_t[0:1, oh * 512:(oh + 1) * 512],
                    start=False, stop=True,
                )
                nc.vector.tensor_copy(
                    vnew_t[:, tc_i * 1024 + oh * 512:tc_i * 1024 + (oh + 1) * 512],
                    vps[tc_i * 2 + oh][:])
        nc.sync.dma_start(
            D["value"].ap().rearrange("(tc p) o -> p tc o", p=128),
            vnew_t[:].rearrange("p (tc o) -> p tc o", tc=4),
        )

    # ---------------- phase A: attention, one head-pair per ns-chunk ----------------
    with ExitStack() as actx:
        kpool = actx.enter_context(tc.tile_pool(name="kpair", bufs=2))
        vpool = actx.enter_context(tc.tile_pool(name="vpair", bufs=2))
        probs_pool = actx.enter_context(tc.tile_pool(name="probs", bufs=3))
        spool = actx.enter_context(tc.tile_pool(name="spsum", bufs=3, space="PSUM"))
        pvpool = actx.enter_context(tc.tile_pool(name="pvpsum", bufs=1, space="PSUM"))
        rpool = actx.enter_context(tc.tile_pool(name="rtiles", bufs=1))

        for c in range(NC_CHUNKS):
            kp = kpool.tile([128, 2048], F32R, name="kp", tag="kp")
            nc.gpsimd.dma_start(kp[:], D["kTc"].ap()[c * 128:(c + 1) * 128, :])
            vp = vpool.tile([128, 2600], F32R, name="vp", tag="vp")
            nc.gpsimd.dma_start(
                vp[:, 0:2080].rearrange("p (j q) -> p j q", q=130),
                D["vaug"].ap().rearrange("(j p) q -> p j q", p=128)
                [:, :, c * 130:(c + 1) * 130],
            )
            # new-token v blocks [v_even|1|v_odd|1] at cols 2080 + tc*130
            nc.gpsimd.dma_start(
                vp[:, 2080:2600].rearrange("p (tc h q) -> p tc h q", h=2, q=65)
                [:, :, :, 64:65],
                onesp_src.rearrange("p (tc h q) -> p tc h q", tc=4, h=2))
            for tc_i in range(4):
                nc.vector.tensor_copy(
                    vp[:, 2080 + tc_i * 130:2080 + (tc_i + 1) * 130]
                    .rearrange("p (h q) -> p h q", q=65)[:, :, 0:64],
                    vnew_t[:, tc_i * 1024 + c * 128:tc_i * 1024 + (c + 1) * 128]
                    .rearrange("p (h q) -> p h q", q=64),
                )

            if c == 1:
                # prefetch Wo while attention runs
                for k in range(8):
                    nc.gpsimd.dma_start(wo_tiles[k][:],
                                        D["Wo"].ap()[k * 128:(k + 1) * 128, :])

            pve = pvpool.tile([65, 512], F32, name="pve", tag="pv_e")
            pvo = pvpool.tile([65, 512], F32, name="pvo", tag="pv_o")
            rhs_e = qT_t[0:64, c * 512:(c + 1) * 512]
            rhs_o = qT_t[64:128, c * 512:(c + 1) * 512]

            for g in range(10):
                se = spool.tile([128, 1024], F32, name="se", tag="sc")
                so = spool.tile([128, 1024], F32, name="so", tag="sc")
                for jj in range(2):
                    j = 2 * g + jj
                    if j < 16:
                        le = kp[0:64, j * 128:(j + 1) * 128]
                        lo = kp[64:128, j * 128:(j + 1) * 128]
                    else:
                        jo = c * 512 + (j - 16) * 128
                        le = knT_t[0:64, jo:jo + 128]
                        lo = knT_t[64:128, jo:jo + 128]
                    nc.tensor.matmul(se[:, jj * 512:(jj + 1) * 512],
                                     lhsT=le, rhs=rhs_e,
                                     start=True, stop=True)
                    nc.tensor.matmul(so[:, jj * 512:(jj + 1) * 512],
                                     lhsT=lo, rhs=rhs_o,
                                     start=True, stop=True)
                pe_t = probs_pool.tile([128, 1024], F32R, name="pe_t", tag="pr")
                nc.scalar.activation(pe_t[:], se[:], ACTF.Exp)
                po_t = probs_pool.tile([128, 1024], F32R, name="po_t", tag="pr")
                nc.scalar.activation(po_t[:], so[:], ACTF.Exp)
                for jj in range(2):
                    j = 2 * g + jj
                    nc.tensor.matmul(
                        pve[:],
                        lhsT=vp[:, j * 130:j * 130 + 65],
                        rhs=pe_t[:, jj * 512:(jj + 1) * 512],
                        start=(j == 0), stop=(j == SCN - 1))
                    nc.tensor.matmul(
                        pvo[:],
                        lhsT=vp[:, j * 130 + 65:(j + 1) * 130],
                        rhs=po_t[:, jj * 512:(jj + 1) * 512],
                        start=(j == 0), stop=(j == SCN - 1))

            # normalize: rows 0..63 = o^T unnormalized, row 64 = denom
            rde = rpool.tile([65, 512], F32, name="rde", tag="rd")
            nc.vector.reciprocal(rde[64:65, :], pve[64:65, :])
            rd0e = rpool.tile([1, 512], F32, name="rd0e", tag="rd0e")
            nc.sync.dma_start(rd0e[:], rde[64:65, :])  # move to physical part 0
            rbe = rpool.tile([64, 512], F32, name="rbe", tag="rb")
            nc.gpsimd.partition_broadcast(rbe[:], rd0e[:])
            nc.vector.tensor_mul(wv_tiles[c][0:64, :], pve[0:64, :], rbe[:])

            rdo = rpool.tile([65, 512], F32, name="rdo", tag="rd")
            nc.vector.reciprocal(rdo[64:65, :], pvo[64:65, :])
            rd0o = rpool.tile([1, 512], F32, name="rd0o", tag="rd0o")
            nc.sync.dma_start(rd0o[:], rdo[64:65, :])
            rbo = rpool.tile([64, 512], F32, name="rbo", tag="rb")
            nc.gpsimd.partition_broadcast(rbo[:], rd0o[:])
            tmo = rpool.tile([64, 512], F32R, name="tmo", tag="tm")
            nc.vector.tensor_mul(tmo[:], pvo[0:64, :], rbo[:])
            # partition shift 0..63 -> 64..127 via SBUF->SBUF DMA
            nc.sync.dma_start(wv_tiles[c][64:128, :], tmo[:])
            if c == 0:
                nc.gpsimd.dma_start(D["dbg"].ap(), wv_tiles[0][:])

    # ---------------- phase O: output projection ----------------
    with ExitStack() as octx:
        opool = octx.enter_context(tc.tile_pool(name="opsum", bufs=2, space="PSUM"))
        for m in range(8):
            po = opool.tile([128, 512], F32, name="po", tag="po")
            for c in range(8):
                nc.tensor.matmul(
                    po[:],
                    lhsT=wo_tiles[c][:, m * 128:(m + 1) * 128],
                    rhs=wv_tiles[c][:],
                    start=(c == 0), stop=(c == 7))
            nc.vector.tensor_scalar(
                outT_t[:, m * 512:(m + 1) * 512], po[:], 1.0, bop_t[:, m:m + 1],
                ALU.mult, ALU.add)
        nc.sync.dma_start(
            D["outT"].ap().rearrange("(m p) t -> p m t", p=128),
            outT_t[:].rearrange("p (m t) -> p m t", m=8),
        )


def build():
    nc = bacc.Bacc("TRN2", target_bir_lowering=False, debug=False)
    D = {}
    D["xT"] = nc.dram_tensor("xT", [NS, T], F32, kind="ExternalInput")
    D["kTc"] = nc.dram_tensor("kTc", [NS, S], F32, kind="ExternalInput")
    D["vaug"] = nc.dram_tensor("vaug", [S, NH * 65], F32, kind="ExternalInput")
    for w in ("Wq", "Wk", "Wv", "Wo"):
        D[w] = nc.dram_tensor(w, [NS, NS], F32, kind="ExternalInput")
    D["bqs"] = nc.dram_tensor("bqs", [128, 8], F32, kind="ExternalInput")
    D["bop"] = nc.dram_tensor("bop", [128, 8], F32, kind="ExternalInput")
    D["bv"] = nc.dram_tensor("bv", [1, NS], F32, kind="ExternalInput")
    D["ones"] = nc.dram_tensor("ones", [128, 136], F32, kind="ExternalInput")
    D["outT"] = nc.dram_tensor("outT", [NS, T], F32, kind="ExternalOutput")
    D["keyT"] = nc.dram_tensor("keyT", [NS, T], F32, kind="ExternalOutput")
    D["value"] = nc.dram_tensor("value", [T, NS], F32, kind="ExternalOutput")
    D["dbg"] = nc.dram_tensor("dbg", [128, 512], F32, kind="ExternalOutput")

    with tile.TileContext(nc) as tc:
        with ExitStack() as ctx:
            _emit(ctx, tc, D)
    nc.compile()
    return nc


_NC_CACHE = None


def _get_nc():
    global _NC_CACHE
    if _NC_CACHE is None:
        _NC_CACHE = build()
    return _NC_CACHE


def prep_core_inputs(b, x, kv_cache, Wq, bq, Wk, Wv, bv, Wo, bo):
    xT = np.ascontiguousarray(x[b].T)                      # [NS, T]
    kTc = np.ascontiguousarray(kv_cache[b, 0, 0].T)        # [NS, S]
    vc = kv_cache[b, 0, 1]                                 # [S, NS]
    vaug = np.empty((S, NH * 65), np.float32)
    va = vaug.reshape(S, NH, 65)
    va[:, :, 0:64] = vc.reshape(S, NH, 64)
    va[:, :, 64] = 1.0
    return {
        "xT": xT, "kTc": kTc, "vaug": vaug,
        "Wq": Wq, "Wk": Wk, "Wv": Wv, "Wo": Wo,
        "bqs": np.ascontiguousarray((bq * SCALE2).reshape(8, 128).T),
        "bop": np.ascontiguousarray(bo.reshape(8, 128).T),
        "bv": np.ascontiguousarray(bv[None, :]),
        "ones": np.ones((128, 136), np.float32),
    }


def kernel(x, kv_cache, offset=0, Wq=None, bq=None, Wk=None, Wv=None, bv=None,
           Wo=None, bo=None, trace=False):
    global LAST_EXEC_NS, LAST_RESULTS
    x = np.asarray(x, np.float32)
    kv_cache = np.asarray(kv_cache, np.float32)
    args = [np.asarray(a, np.float32) for a in (Wq, bq, Wk, Wv, bv, Wo, bo)]
    in_maps = [prep_core_inputs(b, x, kv_cache, *args) for b in range(B)]
    nc = _get_nc()
    res = run_bass_kernel_spmd(nc, in_maps, core_ids=list(range(B)), trace=trace)
    LAST_EXEC_NS = res.exec_time_ns
    LAST_RESULTS = res
    out = np.stack([res.results[b]["outT"].T for b in range(B)])
    key = np.stack([res.results[b]["keyT"].T for b in range(B)])
    value = np.stack([res.results[b]["value"] for b in range(B)])
    return (np.ascontiguousarray(out), np.ascontiguousarray(key),
            np.ascontiguousarray(value))



# revision 13
# speedup vs baseline: 1.5262x; 1.5262x over previous
"""Trainium2 Bass kernel: cached multi-head self-attention decoder block.

Per-core (batch-parallel, B=8 -> 8 cores), fully fused single loop:
  - all weights resident in SBUF as bf16 (64KB), loaded via priority DMA
  - projections (q/k/v chunk bursts) interleaved into the attention chunk
    loop as PE filler so the Tensor engine never idles (p-state ramps to
    2.4 GHz)
  - scores: lhsT = k-block [64d, 128s] bf16, rhs = q chunk [64d, 512t]
    -> PSUM f32 [128s, 1024] (2 j-blocks per exp)
  - exp on ACT -> bf16 probs; PV: lhsT = [v|1] aug blocks bf16,
    accumulated over 20 s-blocks -> rows 0..63 = unnormalized o^T,
    row 64 = softmax denominator
  - normalize: DVE reciprocal of denom row + K=1 matmul broadcast of the
    reciprocal across 64 partitions (replaces gpsimd partition_broadcast)
  - output projection as a tail phase; outputs DMA'd as bf16, host upcasts
Host side pre-blocks all DMA sources so every descriptor is >=1KB
contiguous.
"""

import numpy as np
import ml_dtypes
from contextlib import ExitStack

import concourse.bass as bass
import concourse.tile as tile
from concourse import bacc, mybir
from concourse.bass_utils import run_bass_kernel_spmd

F32 = mybir.dt.float32
F32R = mybir.dt.float32r
BF16 = mybir.dt.bfloat16
ALU = mybir.AluOpType
ACTF = mybir.ActivationFunctionType
NPBF = ml_dtypes.bfloat16

B, T, S, NS, NH, HD = 8, 512, 2048, 1024, 16, 64
ST = S + T              # 2560
NC = NS // 128          # 8 chunks (head pairs)
SCN = ST // 128         # 20 s-blocks (16 cache + 4 new)
SCALE2 = float(HD ** -0.5)  # fold both q and k scales into q

LAST_EXEC_NS = None
LAST_RESULTS = None


def _emit(ctx, tc, D):
    nc = tc.nc

    # ---------------- constants / persistent SBUF ----------------
    const = ctx.enter_context(tc.tile_pool(name="const", bufs=1))
    bqs_t = const.tile([128, 8], F32, name="bqs_t")
    bop_t = const.tile([128, 8], F32, name="bop_t")
    bvr_t = const.tile([1, NS], BF16, name="bvr_t")
    ones_bf = const.tile([1, 128], BF16, name="ones_bf")
    ones_fr = const.tile([65, 64], F32R, name="ones_fr")
    nc.vector.memset(ones_bf[:], 1.0)
    nc.gpsimd.dma_start(ones_fr[:], D["onesf"].ap()[:, :])

    xT_t = const.tile([128, 4096], BF16, name="xT_t")  # k-chunk-major cols

    pers = ctx.enter_context(tc.tile_pool(name="pers", bufs=1))
    qT_t = pers.tile([128, 4096], BF16, name="qT_t")    # (x@Wq+bq)*s2, chunk-major
    knT_t = pers.tile([128, 4096], BF16, name="knT_t")  # (x@Wk)^T, chunk-major
    vnew_t = pers.tile([128, 4096], BF16, name="vnew_t")  # x@Wv+bv natural, tc-major
    outT_t = pers.tile([128, 4096], BF16, name="outT_t")

    wv_pool = ctx.enter_context(tc.tile_pool(name="wv", bufs=1))
    wv_tiles = [wv_pool.tile([128, 512], BF16, name=f"wv_{c}", tag=f"wv{c}")
                for c in range(NC)]

    wpool = ctx.enter_context(tc.tile_pool(name="wts", bufs=1))
    wq_t = [wpool.tile([128, 1024], BF16, name=f"wq{k}", tag=f"wq{k}")
            for k in range(8)]
    wk_t = [wpool.tile([128, 1024], BF16, name=f"wk{k}", tag=f"wk{k}")
            for k in range(8)]
    wvw_t = [wpool.tile([128, 1024], BF16, name=f"wvw{k}", tag=f"wvw{k}")
             for k in range(8)]
    wo_t = [wpool.tile([128, 1024], BF16, name=f"wo{k}", tag=f"wo{k}")
            for k in range(8)]

    kpool = ctx.enter_context(tc.tile_pool(name="kpair", bufs=2))
    vpool = ctx.enter_context(tc.tile_pool(name="vpair", bufs=2))
    probs = ctx.enter_context(tc.tile_pool(name="probs", bufs=6))
    rpool = ctx.enter_context(tc.tile_pool(name="recip", bufs=2))
    tmpool = ctx.enter_context(tc.tile_pool(name="tmo", bufs=2))
    sspool = ctx.enter_context(tc.tile_pool(name="ssb", bufs=2))

    spool = ctx.enter_context(tc.tile_pool(name="spsum", bufs=2, space="PSUM"))
    pvpool = ctx.enter_context(tc.tile_pool(name="pvpsum", bufs=1, space="PSUM"))
    fpool = ctx.enter_context(tc.tile_pool(name="fill", bufs=2, space="PSUM"))

    # ---------------- DMA loads in priority order (Pool queue) ----------
    nc.gpsimd.dma_start(bqs_t[:], D["bqs"][:, :])
    nc.gpsimd.dma_start(bop_t[:], D["bop"][:, :])
    nc.gpsimd.dma_start(bvr_t[:], D["bvr"][:, :])
    nc.gpsimd.dma_start(xT_t[:], D["xT"].ap()[:, :])
    for k in range(8):
        nc.gpsimd.dma_start(wq_t[k][:], D["Wq"].ap()[k * 128:(k + 1) * 128, :])

    kp_tiles = [None] * NC
    vp_tiles = [None] * NC

    def prefetch(c):
        kp = kpool.tile([128, 2048], BF16, name="kp", tag="kp")
        nc.gpsimd.dma_start(kp[:], D["kTc"].ap()[c * 128:(c + 1) * 128, :])
        vp = vpool.tile([128, 2600], BF16, name="vp", tag="vp")
        nc.gpsimd.dma_start(vp[:, 0:2080], D["vb"].ap()[c * 128:(c + 1) * 128, :])
        # ones slots of the new-token aug blocks
        nc.vector.memset(
            vp[:, 2080:2600].rearrange("p (tc h q) -> p tc h q", h=2, q=65)
            [:, :, :, 64:65], 1.0)
        kp_tiles[c], vp_tiles[c] = kp, vp

    prefetch(0)
    for k in range(8):
        nc.gpsimd.dma_start(wk_t[k][:], D["Wk"].ap()[k * 128:(k + 1) * 128, :])
    for k in range(8):
        nc.gpsimd.dma_start(wvw_t[k][:], D["Wv"].ap()[k * 128:(k + 1) * 128, :])
    prefetch(1)
    for k in range(8):
        nc.gpsimd.dma_start(wo_t[k][:], D["Wo"].ap()[k * 128:(k + 1) * 128, :])

    # ---------------- burst helpers (filler PE work) ----------------
    def q_burst(m):
        pt = fpool.tile([128, 512], F32, name=f"qp{m}", tag="f")
        for k in range(8):
            nc.tensor.matmul(pt[:], lhsT=wq_t[k][:, m * 128:(m + 1) * 128],
                             rhs=xT_t[:, k * 512:(k + 1) * 512],
                             start=(k == 0), stop=(k == 7))
        nc.vector.tensor_scalar(qT_t[:, m * 512:(m + 1) * 512], pt[:],
                                SCALE2, bqs_t[:, m:m + 1], ALU.mult, ALU.add)

    def k_burst(c):
        pt = fpool.tile([128, 512], F32, name=f"kp{c}", tag="f")
        for k in range(8):
            nc.tensor.matmul(pt[:], lhsT=wk_t[k][:, c * 128:(c + 1) * 128],
                             rhs=xT_t[:, k * 512:(k + 1) * 512],
                             start=(k == 0), stop=(k == 7))
        nc.vector.tensor_copy(knT_t[:, c * 512:(c + 1) * 512], pt[:])
        nc.sync.dma_start(D["keyT"].ap()[c * 128:(c + 1) * 128, :],
                          knT_t[:, c * 512:(c + 1) * 512])

    def v_burst4(cg, t4):
        # value[t4-block, chunks 4cg..4cg+3] — one accumulation group per
        # PSUM bank (start=True clears the WHOLE bank, so no sub-regions)
        pt = fpool.tile([128, 512], F32, name=f"vb{cg}{t4}", tag="f")
        for k in range(8):
            nc.tensor.matmul(
                pt[:],
                lhsT=xT_t[:, k * 512 + t4 * 128:k * 512 + (t4 + 1) * 128],
                rhs=wvw_t[k][:, cg * 512:(cg + 1) * 512],
                start=(k == 0), stop=False)
        nc.tensor.matmul(pt[:], lhsT=ones_bf[0:1, 0:128],
                         rhs=bvr_t[0:1, cg * 512:(cg + 1) * 512],
                         start=False, stop=True)
        nc.vector.tensor_copy(
            vnew_t[:, t4 * 1024 + cg * 512:t4 * 1024 + (cg + 1) * 512], pt[:])

    def vp_aug(c):
        vp = vp_tiles[c]
        for t4 in range(4):
            base = 2080 + t4 * 130
            src = t4 * 1024 + c * 128
            nc.vector.tensor_copy(vp[:, base:base + 64],
                                  vnew_t[:, src:src + 64])
            nc.vector.tensor_copy(vp[:, base + 65:base + 129],
                                  vnew_t[:, src + 64:src + 128])

    # normalize split: DVE recips at end of chunk c; PE scale matmuls +
    # DVE muls early in chunk c+1 (or tail for c=7)
    chunk_state = {}

    def recips(c, pve, pvo):
        rt = rpool.tile([65, 1024], F32R, name=f"rt{c}", tag="rt")
        with nc.allow_low_precision(reason="f32r rounding for scale bcast"):
            nc.vector.reciprocal(rt[64:65, 0:512], pve[64:65, :])
            nc.vector.reciprocal(rt[64:65, 512:1024], pvo[64:65, :])
        chunk_state[c] = (rt, pve, pvo)

    def normalize(c):
        rt, pve, pvo = chunk_state.pop(c)
        sce = fpool.tile([128, 512], F32, name=f"sce{c}", tag="f")
        nc.tensor.matmul(sce[0:64, :], lhsT=ones_fr[64:65, :],
                         rhs=rt[64:65, 0:512], start=True, stop=True)
        sco = fpool.tile([128, 512], F32, name=f"sco{c}", tag="f")
        nc.tensor.matmul(sco[0:64, :], lhsT=ones_fr[64:65, :],
                         rhs=rt[64:65, 512:1024], start=True, stop=True)
        # DVE can read only one PSUM operand: stage scale tiles in SBUF
        ssb = sspool.tile([64, 1024], F32, name=f"ssb{c}", tag="ssb")
        nc.vector.tensor_copy(ssb[:, 0:512], sce[0:64, :])
        nc.vector.tensor_copy(ssb[:, 512:1024], sco[0:64, :])
        nc.vector.tensor_mul(wv_tiles[c][0:64, :], pve[0:64, :],
                             ssb[:, 0:512])
        tm = tmpool.tile([64, 512], BF16, name=f"tm{c}", tag="tm")
        nc.vector.tensor_mul(tm[:], pvo[0:64, :], ssb[:, 512:1024])
        nc.sync.dma_start(wv_tiles[c][64:128, :], tm[:])

    # ---------------- attention chunk ----------------
    def scores_g(c, g, kp):
        se = spool.tile([128, 1024], F32, name="se", tag="sc")
        so = spool.tile([128, 1024], F32, name="so", tag="sc")
        rhs_e = qT_t[0:64, c * 512:(c + 1) * 512]
        rhs_o = qT_t[64:128, c * 512:(c + 1) * 512]
        for jj in range(2):
            j = 2 * g + jj
            if j < 16:
                le = kp[0:64, j * 128:(j + 1) * 128]
            else:
                jo = c * 512 + (j - 16) * 128
                le = knT_t[0:64, jo:jo + 128]
            nc.tensor.matmul(se[:, jj * 512:(jj + 1) * 512], lhsT=le,
                             rhs=rhs_e, start=True, stop=True)
        pe_t = probs.tile([128, 1024], BF16, name="pe", tag="pr")
        nc.scalar.activation(pe_t[:], se[:], ACTF.Exp)
        for jj in range(2):
            j = 2 * g + jj
            if j < 16:
                lo = kp[64:128, j * 128:(j + 1) * 128]
            else:
                jo = c * 512 + (j - 16) * 128
                lo = knT_t[64:128, jo:jo + 128]
            nc.tensor.matmul(so[:, jj * 512:(jj + 1) * 512], lhsT=lo,
                             rhs=rhs_o, start=True, stop=True)
        po_t = probs.tile([128, 1024], BF16, name="po", tag="pr")
        nc.scalar.activation(po_t[:], so[:], ACTF.Exp)
        return pe_t, po_t

    def pv_g(g, pr, vp, pve, pvo):
        pe_t, po_t = pr
        for jj in range(2):
            j = 2 * g + jj
            nc.tensor.matmul(pve[:], lhsT=vp[:, j * 130:j * 130 + 65],
                             rhs=pe_t[:, jj * 512:(jj + 1) * 512],
                             start=(j == 0), stop=(j == SCN - 1))
            nc.tensor.matmul(pvo[:], lhsT=vp[:, j * 130 + 65:(j + 1) * 130],
                             rhs=po_t[:, jj * 512:(jj + 1) * 512],
                             start=(j == 0), stop=(j == SCN - 1))

    # ---------------- head: q chunk 0 ----------------
    q_burst(0)

    # ---------------- main chunk loop ----------------
    # filler PE bursts per chunk (consumed one per slot between groups)
    chunk_fillers = {
        0: [lambda: q_burst(1), lambda: k_burst(0),
            lambda: v_burst4(0, 0), lambda: v_burst4(0, 1),
            lambda: v_burst4(0, 2), lambda: v_burst4(0, 3),
            lambda: vp_aug(0)],
        1: [lambda: q_burst(2), lambda: k_burst(1),
            lambda: v_burst4(1, 0), lambda: v_burst4(1, 1),
            lambda: vp_aug(1)],
        2: [lambda: q_burst(3), lambda: k_burst(2),
            lambda: v_burst4(1, 2), lambda: v_burst4(1, 3),
            lambda: vp_aug(2)],
        3: [lambda: q_burst(4), lambda: k_burst(3), lambda: vp_aug(3)],
        4: [lambda: q_burst(5), lambda: k_burst(4), lambda: vp_aug(4)],
        5: [lambda: q_burst(6), lambda: k_burst(5), lambda: vp_aug(5)],
        6: [lambda: q_burst(7), lambda: k_burst(6), lambda: vp_aug(6)],
        7: [lambda: k_burst(7), lambda: vp_aug(7)],
    }

    for c in range(NC):
        kp, vp = kp_tiles[c], vp_tiles[c]
        pr = [None] * 10
        fills = iter(chunk_fillers[c])

        def fill():
            f = next(fills, None)
            if f is not None:
                f()

        fill()  # q burst (k burst for c=7)
        # normalize(c-1) reads the old pve/pvo buffers; its reads must be
        # emitted BEFORE this chunk's pvpool allocation reuses them
        if c > 0:
            normalize(c - 1)
        pve = pvpool.tile([65, 512], F32, name="pve", tag="pv_e")
        pvo = pvpool.tile([65, 512], F32, name="pvo", tag="pv_o")
        pr[0] = scores_g(c, 0, kp)
        fill()  # k burst
        pr[1] = scores_g(c, 1, kp)
        fill()
        pr[2] = scores_g(c, 2, kp)
        pv_g(0, pr[0], vp, pve, pvo)
        fill()
        pr[3] = scores_g(c, 3, kp)
        pv_g(1, pr[1], vp, pve, pvo)
        fill()
        pr[4] = scores_g(c, 4, kp)
        pv_g(2, pr[2], vp, pve, pvo)
        fill()
        pr[5] = scores_g(c, 5, kp)
        pv_g(3, pr[3], vp, pve, pvo)
        fill()
        for g in range(6, 10):
            pr[g] = scores_g(c, g, kp)
            pv_g(g - 2, pr[g - 2], vp, pve, pvo)
        pv_g(8, pr[8], vp, pve, pvo)
        pv_g(9, pr[9], vp, pve, pvo)
        recips(c, pve, pvo)
        # prefetch after ALL reads of this chunk's kp/vp are emitted (the
        # new DMA reuses their buffers via the bufs=2 rotation)
        if c + 2 < NC:
            prefetch(c + 2)

    # ---------------- tail: last normalize + output projection ----------
    normalize(NC - 1)
    nc.sync.dma_start(
        D["value"].ap().rearrange("(tc p) o -> p tc o", p=128),
        vnew_t[:].rearrange("p (tc o) -> p tc o", tc=4))
    for m in range(8):
        pt = fpool.tile([128, 512], F32, name=f"op{m}", tag="f")
        for cc in range(8):
            nc.tensor.matmul(pt[:], lhsT=wo_t[cc][:, m * 128:(m + 1) * 128],
                             rhs=wv_tiles[cc][:], start=(cc == 0),
                             stop=(cc == 7))
        nc.vector.tensor_scalar(outT_t[:, m * 512:(m + 1) * 512], pt[:],
                                1.0, bop_t[:, m:m + 1], ALU.mult, ALU.add)
    nc.sync.dma_start(
        D["outT"].ap().rearrange("(m p) t -> p m t", p=128),
        outT_t[:].rearrange("p (m t) -> p m t", m=8))


def build():
    nc = bacc.Bacc("TRN2", target_bir_lowering=False, debug=False)
    D = {}
    D["xT"] = nc.dram_tensor("xT", [128, 4096], BF16, kind="ExternalInput")
    D["kTc"] = nc.dram_tensor("kTc", [NS, S], BF16, kind="ExternalInput")
    D["vb"] = nc.dram_tensor("vb", [NC * 128, 2080], BF16, kind="ExternalInput")
    for w in ("Wq", "Wk", "Wv", "Wo"):
        D[w] = nc.dram_tensor(w, [NS, NS], BF16, kind="ExternalInput")
    D["bqs"] = nc.dram_tensor("bqs", [128, 8], F32, kind="ExternalInput")
    D["bop"] = nc.dram_tensor("bop", [128, 8], F32, kind="ExternalInput")
    D["bvr"] = nc.dram_tensor("bvr", [1, NS], BF16, kind="ExternalInput")
    D["onesf"] = nc.dram_tensor("onesf", [65, 64], F32, kind="ExternalInput")
    D["outT"] = nc.dram_tensor("outT", [NS, T], BF16, kind="ExternalOutput")
    D["keyT"] = nc.dram_tensor("keyT", [NS, T], BF16, kind="ExternalOutput")
    D["value"] = nc.dram_tensor("value", [T, NS], BF16, kind="ExternalOutput")

    with tile.TileContext(nc) as tc:
        with ExitStack() as ctx:
            _emit(ctx, tc, D)
    nc.compile()
    return nc


_NC_CACHE = None


def _get_nc():
    global _NC_CACHE
    if _NC_CACHE is None:
        _NC_CACHE = build()
    return _NC_CACHE


def prep_core_inputs(b, x, kv_cache, WqB, WkB, WvB, WoB, bqs, bop, bvr):
    xT = np.ascontiguousarray(x[b].T).reshape(8, 128, 512) \
        .transpose(1, 0, 2).reshape(128, 4096).astype(NPBF)
    kTc = np.ascontiguousarray(kv_cache[b, 0, 0].T).astype(NPBF)  # [NS, S]
    vjp = kv_cache[b, 0, 1].reshape(16, 128, NH, HD)  # [j, p, h, d]
    vh = vjp.transpose(2, 1, 0, 3)                    # [h, p, j, d]
    vb = np.ones((NC, 128, 16, 130), NPBF)
    vb[..., 0:64] = vh[0::2]
    vb[..., 65:129] = vh[1::2]
    return {
        "xT": xT, "kTc": kTc, "vb": vb.reshape(NC * 128, 2080),
        "Wq": WqB, "Wk": WkB, "Wv": WvB, "Wo": WoB,
        "bqs": bqs, "bop": bop, "bvr": bvr,
        "onesf": np.ones((65, 64), np.float32),
    }


def kernel(x, kv_cache, offset=0, Wq=None, bq=None, Wk=None, Wv=None, bv=None,
           Wo=None, bo=None, trace=False):
    global LAST_EXEC_NS, LAST_RESULTS
    x = np.asarray(x, np.float32)
    kv_cache = np.asarray(kv_cache, np.float32)
    Wq, bq, Wk, Wv, bv, Wo, bo = [np.asarray(a, np.float32)
                                  for a in (Wq, bq, Wk, Wv, bv, Wo, bo)]
    WqB, WkB, WvB, WoB = [w.astype(NPBF) for w in (Wq, Wk, Wv, Wo)]
    bqs = np.ascontiguousarray((bq * SCALE2).reshape(8, 128).T)
    bop = np.ascontiguousarray(bo.reshape(8, 128).T)
    bvr = bv[None, :].astype(NPBF)
    in_maps = [prep_core_inputs(b, x, kv_cache, WqB, WkB, WvB, WoB,
                                bqs, bop, bvr) for b in range(B)]
    nc = _get_nc()
    res = run_bass_kernel_spmd(nc, in_maps, core_ids=list(range(B)), trace=trace)
    LAST_EXEC_NS = res.exec_time_ns
    LAST_RESULTS = res
    out = np.stack([res.results[b]["outT"].astype(np.float32).T
                    for b in range(B)])
    key = np.stack([res.results[b]["keyT"].astype(np.float32).T
                    for b in range(B)])
    value = np.stack([res.results[b]["value"].astype(np.float32)
                      for b in range(B)])
    return (np.ascontiguousarray(out), np.ascontiguousarray(key),
            np.ascontiguousarray(value))
